# revision 16
# baseline (speedup 1.0000x reference)
"""Distributed brute-force kNN (retrieval) kernel for 8 Trainium2 NeuronCores.

Strategy (standard distributed IVF-flat pattern):
  - Shard the datastore X_train row-wise across 8 cores (25000 rows each).
  - Each core computes approximate neg-scores s[q,n] = 2*q.x_n - (|x_n|^2-768)
    for all 256 queries against its shard.  The PE does the 2qx matmul in
    fp8(e4m3) DoubleRowSwInterleave mode (K=768 as 3 double-pumped chunks
    of 256 with host-interleaved weights) plus one bf16 K=2 matmul folding
    in the exactly-split centered -|x|^2 term.  Engines then pipeline:
    ScalarE copies each 2-bank psum tile to bf16 SBUF, and the DVE selects
    the top-8 (value+index) per 1024-wide chunk on all-16-bit operands
    (bf16 values, u16 indices - the DVE 2x packed mode).
  - Two psum tiles per query-half, explicitly ping-ponged across chunks
    (depth-2 software pipeline): PE computes chunk N+1 while ScalarE/DVE
    drain chunk N.
  - Host merges the 8x200 candidates per query, takes the approximate
    top-128, recomputes exact fp32 distances for only those 128 (0.03% of
    the FLOPs), and applies the exact [256,32] linear + prefix-softmax
    epilogue.

  Safety: a true top-32 member is lost only if fp8 score noise (sigma~2.1)
  demotes it below rank 8 in its 1024-chunk or below rank 128 globally.
  Measured on this dataset with wider 2048 chunks: worst chunk-rank 3,
  worst global rank 49.  max8/max_index assign ties distinct indices, so
  bf16 value collisions cannot drop candidates.
"""

import sys

try:
    import concourse.bacc  # noqa: F401
except ImportError:  # toolchain lives here in the eval container
    sys.path.insert(0, "/opt/trn_rl_repo")

import ml_dtypes
import numpy as np

import concourse.bacc as bacc
import concourse.mybir as mybir
import concourse.tile as tile
from concourse.bass_utils import run_bass_kernel_spmd

# Problem geometry (fixed by the task)
B = 256          # queries
D = 768          # embedding dim
N = 200000       # datastore rows
M = 8            # cores
NS = N // M      # rows per core = 25000
KCH = D // 128   # K chunks of 128 = 6
KP = KCH // 2    # DoubleRow K-pair chunks = 3
CW = 1024        # selection chunk width = one 2-bank psum tile
JW = 512         # moving-operand slice (walrus s3d3 cap)
NCH = (NS + CW - 1) // CW               # 25 chunks (24x1024 + 1x424)
NCAND = NCH * 8                         # level-1 candidates/query/core = 200
KK = 32          # top-k
RESCUE = 128     # approx candidates refined exactly on host
X2C = 768.0      # |x|^2 centering constant (E[|x|^2] for unit gaussians)

_PROGRAM = None


def _build_program(repeat=1):
    """Build + compile the per-core Bass program once.

    repeat>1 wraps the compute body in an on-device loop (for timing only).
    """
    nc = bacc.Bacc("TRN2", target_bir_lowering=False, debug=False, num_devices=M)
    f32 = mybir.dt.float32
    bf16 = mybir.dt.bfloat16
    f8 = mybir.dt.float8e4
    u16 = mybir.dt.uint16

    xt = nc.dram_tensor("xt", [D, NS], f8, kind="ExternalInput").ap()
    x2b = nc.dram_tensor("x2b", [128, NS], bf16, kind="ExternalInput").ap()
    q2t = nc.dram_tensor("q2t", [KP * 128, 2 * B], f8, kind="ExternalInput").ap()
    v1o = nc.dram_tensor("v1", [B, NCAND], bf16, kind="ExternalOutput").ap()
    i1o = nc.dram_tensor("i1", [B, NCAND], u16, kind="ExternalOutput").ap()

    xt_r = xt.rearrange("(c p) n -> p c n", p=128)    # [128, 6, 25000]
    q2t_r = q2t.rearrange("(c p) q -> p c q", p=128)  # [128, 3, 512] interleaved

    with tile.TileContext(nc) as tc:
        with (
            tc.tile_pool(name="const", bufs=1) as cpool,
            tc.tile_pool(name="xt", bufs=5) as xpool,
            tc.tile_pool(name="psum", bufs=1, space="PSUM") as ppool,
            tc.tile_pool(name="sub", bufs=1) as spool,
            tc.tile_pool(name="cand", bufs=1) as candpool,
        ):
            q2t_sb = cpool.tile([128, KP, 2 * B], f8)
            nc.sync.dma_start(q2t_sb[:, :, :], q2t_r)
            x2_sb = cpool.tile([128, NS], bf16)
            nc.sync.dma_start(x2_sb[:, :], x2b)

            v1 = [candpool.tile([128, NCAND], bf16, name=f"v1_{qt}")
                  for qt in range(2)]
            i1 = [candpool.tile([128, NCAND], u16, name=f"i1_{qt}")
                  for qt in range(2)]
            # explicit depth-2 ping-pong: [parity][qt], 2-bank psum tiles
            pss = [[ppool.tile([128, CW], f32, name=f"ps{par}{qt}")
                    for qt in range(2)] for par in range(2)]
            sbs = [[spool.tile([128, CW], bf16, name=f"sb{par}{qt}")
                    for qt in range(2)] for par in range(2)]

            import contextlib
            rep_ctx = tc.For_i(0, repeat, 1) if repeat > 1 else contextlib.nullcontext()
            with rep_ctx:
                _emit_body(nc, tc, xpool, pss, sbs, q2t_sb, x2_sb, xt_r,
                           v1, i1)

            for qt in range(2):
                qsl = slice(qt * 128, (qt + 1) * 128)
                nc.sync.dma_start(v1o[qsl, :], v1[qt][:, :])
                nc.sync.dma_start(i1o[qsl, :], i1[qt][:, :])

    nc.compile()
    return nc


def _emit_body(nc, tc, xpool, pss, sbs, q2t_sb, x2_sb, xt_r, v1, i1):
    f8 = mybir.dt.float8e4
    SWIL = mybir.MatmulPerfMode.DoubleRowSwInterleave
    for ch in range(NCH):
        n0 = ch * CW
        w = min(CW, NS - n0)
        par = ch % 2
        xt_sb = xpool.tile([128, KCH, CW], f8, name="xt_sb")
        nc.sync.dma_start(xt_sb[:, :, :w], xt_r[:, :, n0:n0 + w])
        jws = [(j, min(JW, w - j)) for j in range(0, w, JW)]
        for qt in range(2):
            ps = pss[par][qt]
            # weight-contiguous order: both 512-column groups back-to-back
            # under the same stationary (interleaved fp8 DoubleRow) operand
            for c in range(KP):
                lhsT = q2t_sb[:, c, qt * 256:(qt + 1) * 256].rearrange(
                    "p (t m) -> p t m", t=2)
                for j, jw in jws:
                    nc.tensor.matmul(
                        ps[:, j:j + jw],
                        lhsT=lhsT,
                        rhs=xt_sb[:, 2 * c:2 * c + 2, j:j + jw],
                        start=(c == 0),
                        stop=(c == KP - 1),
                        perf_mode=SWIL,
                    )
        # ScalarE downcasts psum to bf16 SBUF (frees the psum tiles for the
        # next same-parity chunk), then the all-16-bit DVE selection scans
        for qt in range(2):
            nc.scalar.copy(out=sbs[par][qt][:, :w], in_=pss[par][qt][:, :w])
        for qt in range(2):
            nc.vector.tensor_tensor(out=sbs[par][qt][:, :w],
                                    in0=sbs[par][qt][:, :w],
                                    in1=x2_sb[:, n0:n0 + w],
                                    op=mybir.AluOpType.subtract)
        for qt in range(2):
            sl = slice(ch * 8, ch * 8 + 8)
            nc.vector.max(out=v1[qt][:, sl], in_=sbs[par][qt][:, :w])
            nc.vector.max_index(out=i1[qt][:, sl], in_max=v1[qt][:, sl],
                                in_values=sbs[par][qt][:, :w])


def get_program():
    global _PROGRAM
    if _PROGRAM is None:
        _PROGRAM = _build_program()
    return _PROGRAM


def _bf16(a):
    return np.asarray(a, np.float32).astype(ml_dtypes.bfloat16)


def _f8(a):
    return np.clip(np.asarray(a, np.float32), -240.0, 240.0).astype(
        ml_dtypes.float8_e4m3)


def _q2_interleave(q8):
    """[768,256] fp8 -> [384,512] DoubleRowSwInterleave weight layout.

    raw free position qt*256 + 2j+i holds W_{K-group i}[col 127-j] (pairs
    interleaved, columns reversed) for the cpair covering K-chunks 2c,2c+1.
    """
    K = np.asarray(q8).reshape(KCH, 128, B)          # [k, p, q]
    out = np.empty((KP, 128, 2 * B), dtype=q8.dtype)
    for c in range(KP):
        for qt in range(2):
            A = K[2 * c, :, qt * 128:(qt + 1) * 128][:, ::-1]
            Bm = K[2 * c + 1, :, qt * 128:(qt + 1) * 128][:, ::-1]
            blk = out[c, :, qt * 256:(qt + 1) * 256]
            blk[:, 0::2] = A
            blk[:, 1::2] = Bm
    return np.ascontiguousarray(out.reshape(KP * 128, 2 * B))


def prep_inputs(queries, X_train):
    """Host-side shard prep: per-core input maps."""
    q2t = _q2_interleave(_f8(2.0 * queries).T)                  # [384,512] fp8
    in_maps = []
    for c in range(M):
        rows = X_train[c * NS:(c + 1) * NS]
        xt_c = np.ascontiguousarray(_f8(rows).T)                # [768, 25000]
        x2_c = np.einsum("nd,nd->n", rows, rows).astype(np.float32) - np.float32(X2C)
        x2b = np.broadcast_to(_bf16(x2_c), (128, NS)).copy()    # [128, 25000]
        in_maps.append({"xt": xt_c, "x2b": x2b, "q2t": q2t})
    return in_maps


def host_finish(results, queries, query_sys, X_train, Y_train, sys_train,
                W, b, max_k):
    """Merge approx candidates, refine top-RESCUE exactly, run the epilogue."""
    base_vals = ((np.arange(NCAND, dtype=np.int64) >> 3) * CW)       # chunk bases
    negs_all = np.concatenate(
        [r["v1"].astype(np.float32) for r in results], axis=1)       # [256, 1600]
    gidx_all = np.concatenate(
        [r["i1"].astype(np.int64) + base_vals[None, :] + c * NS
         for c, r in enumerate(results)], axis=1)
    part = np.argpartition(-negs_all, RESCUE, axis=1)[:, :RESCUE]
    cand = np.take_along_axis(gidx_all, part, axis=1)                # [256, 128]

    # exact fp32 refinement of the surviving candidates only
    q2 = np.einsum("qd,qd->q", queries, queries).astype(np.float32)
    Xs = X_train[cand]                                               # [256,128,768]
    qx = np.einsum("qd,qkd->qk", queries, Xs).astype(np.float32)
    x2s = np.einsum("qkd,qkd->qk", Xs, Xs).astype(np.float32)
    d2c = q2[:, None] + x2s - 2.0 * qx                               # [256, 128]

    ordr = np.argsort(d2c, axis=1, kind="stable")[:, :max_k]
    D2 = np.take_along_axis(d2c, ordr, axis=1)                       # [256, 32]
    I = np.take_along_axis(cand, ordr, axis=1)

    scores = Y_train[I]
    res_sys = sys_train[I]
    local = res_sys == query_sys[:, None]
    loc = D2[..., None] * W[:, 0] + b                                # [256,32,2]
    new_D = np.where(local, loc[..., 1], loc[..., 0]).astype(np.float32)

    neg = -new_D
    m = np.max(neg, axis=-1, keepdims=True)
    w = np.exp(neg - m)
    num = np.cumsum(w * scores, axis=-1)
    den = np.cumsum(w, axis=-1)
    with np.errstate(invalid="ignore", divide="ignore"):
        knns_scores = (num / den).astype(np.float32)
    return new_D, knns_scores


def kernel(queries, query_sys, X_train, Y_train, sys_train, W, b, max_k):
    queries = np.asarray(queries, dtype=np.float32)
    query_sys = np.asarray(query_sys, dtype=np.int32)
    X_train = np.asarray(X_train, dtype=np.float32)
    Y_train = np.asarray(Y_train, dtype=np.float32)
    sys_train = np.asarray(sys_train, dtype=np.int32)
    W = np.asarray(W, dtype=np.float32)
    b = np.asarray(b, dtype=np.float32)
    max_k = int(max_k)
    assert max_k == KK, f"kernel hardcodes k=32, got {max_k}"
    assert queries.shape == (B, D) and X_train.shape == (N, D)

    nc = get_program()
    in_maps = prep_inputs(queries, X_train)
    res = run_bass_kernel_spmd(nc, in_maps, core_ids=list(range(M)))
    return host_finish(res.results, queries, query_sys, X_train, Y_train,
                       sys_train, W, b, max_k)


# revision 17
# speedup vs baseline: 1.3002x; 1.3002x over previous
"""Distributed brute-force kNN (retrieval) kernel for 8 Trainium2 NeuronCores.

Strategy (standard distributed IVF-flat pattern):
  - Shard the datastore X_train row-wise across 8 cores (25000 rows each).
  - Each core computes approximate neg-scores s[q,n] = 2*q.x_n - (|x_n|^2-768)
    for all 256 queries against its shard.  The PE does the 2qx matmul in
    fp8(e4m3) DoubleRowSwInterleave mode (K=768 as 3 double-pumped chunks
    of 256 with host-interleaved weights) plus one bf16 K=2 matmul folding
    in the exactly-split centered -|x|^2 term.  Engines then pipeline:
    ScalarE copies each 2-bank psum tile to bf16 SBUF, and the DVE selects
    the top-8 (value+index) per 1024-wide chunk on all-16-bit operands
    (bf16 values, u16 indices - the DVE 2x packed mode).
  - Two psum tiles per query-half, explicitly ping-ponged across chunks
    (depth-2 software pipeline): PE computes chunk N+1 while ScalarE/DVE
    drain chunk N.
  - Host merges the 8x200 candidates per query, takes the approximate
    top-128, recomputes exact fp32 distances for only those 128 (0.03% of
    the FLOPs), and applies the exact [256,32] linear + prefix-softmax
    epilogue.

  Safety: a true top-32 member is lost only if fp8 score noise (sigma~2.1)
  demotes it below rank 8 in its 1024-chunk or below rank 128 globally.
  Measured on this dataset with wider 2048 chunks: worst chunk-rank 3,
  worst global rank 49.  max8/max_index assign ties distinct indices, so
  bf16 value collisions cannot drop candidates.
"""

import sys

try:
    import concourse.bacc  # noqa: F401
except ImportError:  # toolchain lives here in the eval container
    sys.path.insert(0, "/opt/trn_rl_repo")

import ml_dtypes
import numpy as np

import concourse.bacc as bacc
import concourse.mybir as mybir
import concourse.tile as tile
from concourse.bass_utils import run_bass_kernel_spmd

# Problem geometry (fixed by the task)
B = 256          # queries
D = 768          # embedding dim
N = 200000       # datastore rows
M = 8            # cores
NS = N // M      # rows per core = 25000
KCH = D // 128   # K chunks of 128 = 6
KP = KCH // 2    # DoubleRow K-pair chunks = 3
CW = 1024        # selection chunk width = one 2-bank psum tile
JW = 512         # moving-operand slice (walrus s3d3 cap)
NCH = (NS + CW - 1) // CW               # 25 chunks (24x1024 + 1x424)
NCAND = NCH * 8                         # level-1 candidates/query/core = 200
KK = 32          # top-k
RESCUE = 128     # approx candidates refined exactly on host
X2C = 768.0      # |x|^2 centering constant (E[|x|^2] for unit gaussians)

_PROGRAM = None


def _build_program(repeat=1):
    """Build + compile the per-core Bass program once.

    repeat>1 wraps the compute body in an on-device loop (for timing only).
    """
    nc = bacc.Bacc("TRN2", target_bir_lowering=False, debug=False, num_devices=M)
    f32 = mybir.dt.float32
    bf16 = mybir.dt.bfloat16
    f8 = mybir.dt.float8e4
    u16 = mybir.dt.uint16

    xt = nc.dram_tensor("xt", [D, NS], f8, kind="ExternalInput").ap()
    x2 = nc.dram_tensor("x2", [2, NS], bf16, kind="ExternalInput").ap()
    q2t = nc.dram_tensor("q2t", [KP * 128, 2 * B], f8, kind="ExternalInput").ap()
    v1o = nc.dram_tensor("v1", [B, NCAND], bf16, kind="ExternalOutput").ap()
    i1o = nc.dram_tensor("i1", [B, NCAND], u16, kind="ExternalOutput").ap()

    xt_r = xt.rearrange("(c p) n -> p c n", p=128)    # [128, 6, 25000]
    q2t_r = q2t.rearrange("(c p) q -> p c q", p=128)  # [128, 3, 512] interleaved

    with tile.TileContext(nc) as tc:
        with (
            tc.tile_pool(name="const", bufs=1) as cpool,
            tc.tile_pool(name="xt", bufs=5) as xpool,
            tc.tile_pool(name="psum", bufs=1, space="PSUM") as ppool,
            tc.tile_pool(name="sub", bufs=1) as spool,
            tc.tile_pool(name="cand", bufs=1) as candpool,
        ):
            q2t_sb = cpool.tile([128, KP, 2 * B], f8)
            nc.sync.dma_start(q2t_sb[:, :, :], q2t_r)
            x2_sb = cpool.tile([2, NS], bf16)
            nc.sync.dma_start(x2_sb[:, :], x2)
            neg1 = cpool.tile([2, 128], bf16)
            nc.vector.memset(neg1[:, :], -1.0)

            v1 = [candpool.tile([128, NCAND], bf16, name=f"v1_{qt}")
                  for qt in range(2)]
            i1 = [candpool.tile([128, NCAND], u16, name=f"i1_{qt}")
                  for qt in range(2)]
            # explicit depth-2 ping-pong: [parity][qt], 2-bank psum tiles
            pss = [[ppool.tile([128, CW], f32, name=f"ps{par}{qt}")
                    for qt in range(2)] for par in range(2)]
            sbs = [[spool.tile([128, CW], bf16, name=f"sb{par}{qt}")
                    for qt in range(2)] for par in range(2)]

            import contextlib
            rep_ctx = tc.For_i(0, repeat, 1) if repeat > 1 else contextlib.nullcontext()
            with rep_ctx:
                _emit_body(nc, tc, xpool, pss, sbs, q2t_sb, x2_sb, xt_r, neg1,
                           v1, i1)

            for qt in range(2):
                qsl = slice(qt * 128, (qt + 1) * 128)
                nc.sync.dma_start(v1o[qsl, :], v1[qt][:, :])
                nc.sync.dma_start(i1o[qsl, :], i1[qt][:, :])

    nc.compile()
    return nc


def _emit_body(nc, tc, xpool, pss, sbs, q2t_sb, x2_sb, xt_r, neg1, v1, i1):
    f8 = mybir.dt.float8e4
    SWIL = mybir.MatmulPerfMode.DoubleRowSwInterleave
    for ch in range(NCH):
        n0 = ch * CW
        w = min(CW, NS - n0)
        par = ch % 2
        xt_sb = xpool.tile([128, KCH, CW], f8, name="xt_sb")
        nc.sync.dma_start(xt_sb[:, :, :w], xt_r[:, :, n0:n0 + w])
        jws = [(j, min(JW, w - j)) for j in range(0, w, JW)]
        # Both qtiles' x2-row matmuls first: they read only resident SBUF,
        # so the in-order PE can execute them while this chunk's xt DMA is
        # still in flight (PE runs matmuls strictly in program order).
        for qt in range(2):
            for j, jw in jws:
                nc.tensor.matmul(
                    pss[par][qt][:, j:j + jw],
                    lhsT=neg1[:, :],
                    rhs=x2_sb[:, n0 + j:n0 + j + jw],
                    start=True,
                    stop=False,
                )
        for qt in range(2):
            ps = pss[par][qt]
            # weight-contiguous order: both 512-column groups back-to-back
            # under the same stationary (interleaved fp8 DoubleRow) operand
            for c in range(KP):
                lhsT = q2t_sb[:, c, qt * 256:(qt + 1) * 256].rearrange(
                    "p (t m) -> p t m", t=2)
                for j, jw in jws:
                    nc.tensor.matmul(
                        ps[:, j:j + jw],
                        lhsT=lhsT,
                        rhs=xt_sb[:, 2 * c:2 * c + 2, j:j + jw],
                        start=False,
                        stop=(c == KP - 1),
                        perf_mode=SWIL,
                    )
        # ScalarE downcasts psum to bf16 SBUF (frees the psum tiles for the
        # next same-parity chunk), then the all-16-bit DVE selection scans
        for qt in range(2):
            nc.scalar.copy(out=sbs[par][qt][:, :w], in_=pss[par][qt][:, :w])
        for qt in range(2):
            sl = slice(ch * 8, ch * 8 + 8)
            nc.vector.max(out=v1[qt][:, sl], in_=sbs[par][qt][:, :w])
            nc.vector.max_index(out=i1[qt][:, sl], in_max=v1[qt][:, sl],
                                in_values=sbs[par][qt][:, :w])


def get_program():
    global _PROGRAM
    if _PROGRAM is None:
        _PROGRAM = _build_program()
    return _PROGRAM


def _bf16(a):
    return np.asarray(a, np.float32).astype(ml_dtypes.bfloat16)


def _f8(a):
    return np.clip(np.asarray(a, np.float32), -240.0, 240.0).astype(
        ml_dtypes.float8_e4m3)


def _q2_interleave(q8):
    """[768,256] fp8 -> [384,512] DoubleRowSwInterleave weight layout.

    raw free position qt*256 + 2j+i holds W_{K-group i}[col 127-j] (pairs
    interleaved, columns reversed) for the cpair covering K-chunks 2c,2c+1.
    """
    K = np.asarray(q8).reshape(KCH, 128, B)          # [k, p, q]
    out = np.empty((KP, 128, 2 * B), dtype=q8.dtype)
    for c in range(KP):
        for qt in range(2):
            A = K[2 * c, :, qt * 128:(qt + 1) * 128][:, ::-1]
            Bm = K[2 * c + 1, :, qt * 128:(qt + 1) * 128][:, ::-1]
            blk = out[c, :, qt * 256:(qt + 1) * 256]
            blk[:, 0::2] = A
            blk[:, 1::2] = Bm
    return np.ascontiguousarray(out.reshape(KP * 128, 2 * B))


def prep_inputs(queries, X_train):
    """Host-side shard prep: per-core input maps."""
    q2t = _q2_interleave(_f8(2.0 * queries).T)                  # [384,512] fp8
    in_maps = []
    for c in range(M):
        rows = X_train[c * NS:(c + 1) * NS]
        xt_c = np.ascontiguousarray(_f8(rows).T)                # [768, 25000]
        x2_c = np.einsum("nd,nd->n", rows, rows).astype(np.float32) - np.float32(X2C)
        x2h = _bf16(x2_c)
        x2l = _bf16(x2_c - x2h.astype(np.float32))
        x2hl = np.ascontiguousarray(np.stack([x2h, x2l]))       # [2, 25000] bf16
        in_maps.append({"xt": xt_c, "x2": x2hl, "q2t": q2t})
    return in_maps


def host_finish(results, queries, query_sys, X_train, Y_train, sys_train,
                W, b, max_k):
    """Merge approx candidates, refine top-RESCUE exactly, run the epilogue."""
    base_vals = ((np.arange(NCAND, dtype=np.int64) >> 3) * CW)       # chunk bases
    negs_all = np.concatenate(
        [r["v1"].astype(np.float32) for r in results], axis=1)       # [256, 1600]
    gidx_all = np.concatenate(
        [r["i1"].astype(np.int64) + base_vals[None, :] + c * NS
         for c, r in enumerate(results)], axis=1)
    part = np.argpartition(-negs_all, RESCUE, axis=1)[:, :RESCUE]
    cand = np.take_along_axis(gidx_all, part, axis=1)                # [256, 128]

    # exact fp32 refinement of the surviving candidates only
    q2 = np.einsum("qd,qd->q", queries, queries).astype(np.float32)
    Xs = X_train[cand]                                               # [256,128,768]
    qx = np.einsum("qd,qkd->qk", queries, Xs).astype(np.float32)
    x2s = np.einsum("qkd,qkd->qk", Xs, Xs).astype(np.float32)
    d2c = q2[:, None] + x2s - 2.0 * qx                               # [256, 128]

    ordr = np.argsort(d2c, axis=1, kind="stable")[:, :max_k]
    D2 = np.take_along_axis(d2c, ordr, axis=1)                       # [256, 32]
    I = np.take_along_axis(cand, ordr, axis=1)

    scores = Y_train[I]
    res_sys = sys_train[I]
    local = res_sys == query_sys[:, None]
    loc = D2[..., None] * W[:, 0] + b                                # [256,32,2]
    new_D = np.where(local, loc[..., 1], loc[..., 0]).astype(np.float32)

    neg = -new_D
    m = np.max(neg, axis=-1, keepdims=True)
    w = np.exp(neg - m)
    num = np.cumsum(w * scores, axis=-1)
    den = np.cumsum(w, axis=-1)
    with np.errstate(invalid="ignore", divide="ignore"):
        knns_scores = (num / den).astype(np.float32)
    return new_D, knns_scores


def kernel(queries, query_sys, X_train, Y_train, sys_train, W, b, max_k):
    queries = np.asarray(queries, dtype=np.float32)
    query_sys = np.asarray(query_sys, dtype=np.int32)
    X_train = np.asarray(X_train, dtype=np.float32)
    Y_train = np.asarray(Y_train, dtype=np.float32)
    sys_train = np.asarray(sys_train, dtype=np.int32)
    W = np.asarray(W, dtype=np.float32)
    b = np.asarray(b, dtype=np.float32)
    max_k = int(max_k)
    assert max_k == KK, f"kernel hardcodes k=32, got {max_k}"
    assert queries.shape == (B, D) and X_train.shape == (N, D)

    nc = get_program()
    in_maps = prep_inputs(queries, X_train)
    res = run_bass_kernel_spmd(nc, in_maps, core_ids=list(range(M)))
    return host_finish(res.results, queries, query_sys, X_train, Y_train,
                       sys_train, W, b, max_k)
